# revision 3
# baseline (speedup 1.0000x reference)
"""Causal self-attention with RoPE on 8 trn2 NeuronCores — v2.

Full inputs -> full output. Sharding: data-parallel on batch (2) x
tensor-parallel on heads (4 heads/core). Each core computes qkv projections,
RoPE, causal attention for its 4 heads, and a partial output projection
(row-parallel slice of Wproj); the host sums the 4 partials per batch.

v2 changes vs v1 (all aimed at PE idle-gap removal; PE is the roofline):
  - causal diagonal masking folded into the scores PSUM accumulation as a
    constant bias matmul (mt^T @ I adds -2^17.58 above the diagonal), so
    exp() output feeds the AV matmul directly: no GPSIMD mask op, no
    Pool->PE dependency chain.
  - RoPE runs in-place in one PSUM bank per pack: proj matmuls accumulate
    ps; DVE writes tsin = ps*sin (SBUF); DVE overwrites ps *= cos; the PE
    rotation matmul accumulates R@tsin onto ps (start=False); one DVE copy
    emits the bf16 pack. Frees 2 PSUM banks vs the v1 rp scheme and
    decouples Q/K-proj PSUM recycling from V/output-proj tiles.
  - PSUM pools: ps (2 banks) / sc (4) / avE+avO (2), with pv/pj sharing
    the ps pool's rotation. No pool oversubscription.
  - copies routed explicitly: V-tile copies on DVE, output-tile copies on
    ACT, keeping the ACT queue otherwise pure exp (v1's 'any' routing let
    copies pollute ACT and stall the sc-bank recycle).
  - normalize: broadcast raw denominators via the DRAM round-trip first,
    then a single [128,T/4] reciprocal on the broadcast tile (halves DVE
    reciprocal work and shortens the chain).
  - softmax skips max-subtraction (scores are ~N(0,1); exp stays well
    inside bf16 range) and gets the denominator from an appended
    ones-column in V.
"""

import math
from functools import lru_cache

import numpy as np

import concourse.bacc as bacc
import concourse.bass as bass
import concourse.mybir as mybir
import concourse.tile as tile
from concourse.bass import ds, ts
from concourse.bass_utils import run_bass_kernel_spmd

B, T, C = 2, 2048, 1024
H, DH = 16, 64
HL = 4  # heads per core
NCORES = 8
ROPE_BASE = 10000.0
MASKB = 196608.0  # causal bias; exactly representable in bf16; /8 -> -24576

F32 = mybir.dt.float32
F32R = mybir.dt.float32r
BF16 = mybir.dt.bfloat16
AF = mybir.ActivationFunctionType
ALU = mybir.AluOpType

TCH = 512  # T chunk (free dim of projection / q chunk of attention)
NJ = T // TCH  # 4
NKT = T // 128  # 16 k tiles


def _build_nc(reps=1, variant=""):
    nc = bacc.Bacc("TRN2", target_bir_lowering=False, debug=False, num_devices=NCORES)

    xT = nc.dram_tensor("xT", [C, T], F32R, kind="ExternalInput")
    wqk = nc.dram_tensor("wqk", [C, 512], F32R, kind="ExternalInput")
    wv = nc.dram_tensor("wv", [C, 256], F32R, kind="ExternalInput")
    wpj = nc.dram_tensor("wpj", [256, C], F32R, kind="ExternalInput")
    r2t = nc.dram_tensor("r2t", [128, 128], BF16, kind="ExternalInput")
    cosp = nc.dram_tensor("cosp", [128, T], F32, kind="ExternalInput")
    sinp = nc.dram_tensor("sinp", [128, T], F32, kind="ExternalInput")
    maskt = nc.dram_tensor("maskt", [128, 128], BF16, kind="ExternalInput")
    mask01t = nc.dram_tensor("mask01t", [128, 128], BF16, kind="ExternalInput")
    identt = nc.dram_tensor("identt", [128, 128], BF16, kind="ExternalInput")
    y = nc.dram_tensor("y", [T, C], F32, kind="ExternalOutput")
    # scratch rows for the denominator partition-broadcast round-trip
    rscr = nc.dram_tensor("rscr", [NJ, 2, 2, TCH], F32, kind="Internal")

    xT_r = xT.rearrange("(co p) t -> p co t", p=128)
    wqk_r = wqk.rearrange("(co p) m -> p co m", p=128)
    wv_r = wv.rearrange("(co p) m -> p co m", p=128)
    wpj_r = wpj.rearrange("(cc p) n -> p cc n", p=128)

    with tile.TileContext(nc) as tc:
        with (
            tc.tile_pool(name="const", bufs=1) as const,
            tc.tile_pool(name="persist", bufs=1) as persist,
            tc.tile_pool(name="work", bufs=3) as work,
            tc.tile_pool(name="expool", bufs=12) as expool,
            tc.tile_pool(name="small", bufs=3) as small,
            tc.tile_pool(name="outst", bufs=8) as outst,
            tc.tile_pool(name="psP", bufs=2, space="PSUM") as psP,
            tc.tile_pool(name="psS", bufs=2, space="PSUM") as psS,
            tc.tile_pool(name="psV", bufs=1, space="PSUM") as psV,
        ):
            # ---- constants ----
            wqk_sb = const.tile([128, 8, 512], F32R)
            wv_sb = const.tile([128, 8, 256], F32R)
            wpj_sb = const.tile([128, 2, 1024], F32R)
            r2t_sb = const.tile([128, 128], BF16)
            mask_sb = const.tile([128, 128], BF16)
            mask01_sb = const.tile([128, 128], BF16)
            ident_sb = const.tile([128, 128], BF16)
            cos_sb = const.tile([128, T], F32)
            sin_sb = const.tile([128, T], F32)
            # first-wave DMAs: only what the first matmuls touch
            for _cc in range(8):
                nc.sync.dma_start(wqk_sb[:, _cc, :], wqk_r[:, _cc, :])
            nc.sync.dma_start(r2t_sb[:], r2t[:])

            # ---- persistent activations ----
            qt_packs = [
                persist.tile([128, T], BF16, tag=f"qt{p}", name=f"qt{p}")
                for p in range(2)
            ]
            kt_packs = [
                persist.tile([128, T], BF16, tag=f"kt{p}", name=f"kt{p}")
                for p in range(2)
            ]
            # V for even local heads: [.., 65] = [v dims | ones]
            vse = persist.tile([128, NKT, 2, 65], BF16, tag="vse")
            # V for odd local heads: [.., 128] = [ones | zeros x63 | v dims]
            vso = persist.tile([128, NKT, 2, 128], BF16, tag="vso")
            ytsb = persist.tile([128, 2, T], F32R, tag="ytsb")

            nc.gpsimd.memset(vse[:, :, :, 64], 1.0)
            nc.gpsimd.memset(vso[:, :, :, 0], 1.0)
            nc.gpsimd.memset(vso[:, :, :, 1:64], 0.0)

            def gen_A(j):
                """Projections + RoPE + V for chunk j. Yields between groups."""
                xt = work.tile([128, 8, TCH], F32R, tag="xt", name=f"xt{j}")
                for _cc in range(8):
                    nc.sync.dma_start(xt[:, _cc, :], xT_r[:, _cc, ts(j, TCH)])
                yield
                for pk in range(4):  # Q pack0, Q pack1, K pack0, K pack1
                    ps = psP.tile([128, TCH], F32, tag="mm", name=f"ps{j}{pk}")
                    for cc in range(4):
                        nc.tensor.matmul(
                            ps[:],
                            wqk_sb[:, cc, ts(pk, 128)],
                            xt[:, cc, :],
                            start=(cc == 0),
                            stop=False,
                        )
                    yield
                    for cc in range(4, 8):
                        nc.tensor.matmul(
                            ps[:],
                            wqk_sb[:, cc, ts(pk, 128)],
                            xt[:, cc, :],
                            start=False,
                            stop=(cc == 7),
                        )
                    yield
                    # rot(q*sin) == rot(q)*sin: sin-multiply straight off PSUM;
                    # the three DVE ops are emitted back-to-back so no other
                    # DVE work lands inside the ps-recycle chain
                    tsin = work.tile([128, TCH], BF16, tag="tsin", name=f"tsn{j}{pk}")
                    nc.vector.tensor_tensor(
                        tsin[:], ps[:], sin_sb[:, ts(j, TCH)], ALU.mult
                    )
                    # in-place: ps *= cos, then accumulate R @ tsin onto it
                    nc.vector.tensor_tensor(
                        ps[:], ps[:], cos_sb[:, ts(j, TCH)], ALU.mult
                    )
                    nc.tensor.matmul(
                        ps[:], r2t_sb[:], tsin[:],
                        start=False, stop=True, skip_group_check=True,
                    )
                    dest = (qt_packs + kt_packs)[pk]
                    nc.vector.tensor_copy(dest[:, ts(j, TCH)], ps[:])
                    yield
                for i in range(4):  # V t-tiles of this chunk
                    kt_i = j * 4 + i
                    pv = psP.tile([128, 256], F32, tag="mm", name=f"pv{j}{i}")
                    for cc in range(4):
                        nc.tensor.matmul(
                            pv[:],
                            xt[:, cc, ts(i, 128)],
                            wv_sb[:, cc, :],
                            start=(cc == 0),
                            stop=False,
                        )
                    yield
                    for cc in range(4, 8):
                        nc.tensor.matmul(
                            pv[:],
                            xt[:, cc, ts(i, 128)],
                            wv_sb[:, cc, :],
                            start=False,
                            stop=(cc == 7),
                        )
                    pv4 = pv[:].rearrange("p (h d) -> p h d", h=4)
                    nc.scalar.copy(vse[:, kt_i, :, 0:64], pv4[:, 0::2, :])
                    nc.vector.tensor_copy(vso[:, kt_i, :, 64:128], pv4[:, 1::2, :])
                    yield

            def gen_B(j):
                """Attention for q chunk j. Yields between kt blocks.

                Scores run one block ahead of the AV matmuls so each exp()
                executes in the shadow of the next block's scores.
                """
                for pp in range(2):  # head pairs (2pp, 2pp+1)
                    KT = kt_packs[pp]
                    QT = qt_packs[pp]
                    n_kt = 4 * j + 4
                    avE = psV.tile([65, TCH], F32, tag="avE", name=f"avE{j}{pp}")
                    avO = psV.tile([128, TCH], F32, tag="avO", name=f"avO{j}{pp}")
                    exs = {}

                    def emit_sc(kt):
                        d = kt - 4 * j
                        col0 = max(d, 0) * 128
                        sc = psS.tile(
                            [128, 2, TCH], F32, tag="sc", name=f"sc{j}{pp}{kt}"
                        )
                        if d >= 0 and "poolmask" in variant:
                            qsl = ds(j * TCH + col0, TCH - col0)
                            nc.tensor.matmul(
                                sc[:, 0, col0:], KT[0:64, ts(kt, 128)],
                                QT[0:64, qsl],
                                start=True, stop=True, tile_position=(0, 0),
                            )
                            nc.tensor.matmul(
                                sc[:, 1, col0:], KT[64:128, ts(kt, 128)],
                                QT[64:128, qsl],
                                start=True, stop=True, tile_position=(64, 0),
                            )
                        elif d >= 0:
                            # causal bias on the diagonal 128-block, then the
                            # two 64-contract head matmuls accumulate onto it
                            asl = slice(col0, col0 + 128)
                            qasl = ds(j * TCH + col0, 128)
                            for h in range(2):
                                nc.tensor.matmul(
                                    sc[:, h, asl],
                                    mask_sb[:],
                                    ident_sb[:],
                                    start=True, stop=False,
                                    skip_group_check=True,
                                )
                                nc.tensor.matmul(
                                    sc[:, h, asl],
                                    KT[64 * h:64 * h + 64, ts(kt, 128)],
                                    QT[64 * h:64 * h + 64, qasl],
                                    start=False, stop=True,
                                    skip_group_check=True,
                                    tile_position=(64 * h, 0),
                                )
                            if col0 + 128 < TCH:
                                bsl = ds(j * TCH + col0 + 128, TCH - col0 - 128)
                                nc.tensor.matmul(
                                    sc[:, 0, col0 + 128:],
                                    KT[0:64, ts(kt, 128)], QT[0:64, bsl],
                                    start=True, stop=True, tile_position=(0, 0),
                                )
                                nc.tensor.matmul(
                                    sc[:, 1, col0 + 128:],
                                    KT[64:128, ts(kt, 128)], QT[64:128, bsl],
                                    start=True, stop=True, tile_position=(64, 0),
                                )
                        else:
                            qsl = ds(j * TCH, TCH)
                            nc.tensor.matmul(
                                sc[:, 0, :], KT[0:64, ts(kt, 128)], QT[0:64, qsl],
                                start=True, stop=True, tile_position=(0, 0),
                            )
                            nc.tensor.matmul(
                                sc[:, 1, :], KT[64:128, ts(kt, 128)], QT[64:128, qsl],
                                start=True, stop=True, tile_position=(64, 0),
                            )
                        ex = expool.tile(
                            [128, 2, TCH], BF16, tag="ex", name=f"ex{j}{pp}{kt}"
                        )
                        if "noexp" not in variant:
                            nc.scalar.activation(
                                ex[:, :, col0:], sc[:, :, col0:], AF.Exp,
                                scale=1.0 / math.sqrt(DH),
                            )
                        if d >= 0 and "poolmask" in variant:
                            msl = slice(col0, col0 + 128)
                            nc.gpsimd.tensor_tensor(
                                ex[:, :, msl],
                                ex[:, :, msl],
                                mask01_sb[:, None, :].to_broadcast((128, 2, 128)),
                                ALU.mult,
                            )
                        exs[kt] = (ex, col0)

                    def emit_av(kt):
                        ex, col0 = exs.pop(kt)
                        st = kt == 0
                        sp = kt == n_kt - 1
                        nc.tensor.matmul(
                            avE[:, col0:], vse[:, kt, pp, :], ex[:, 0, col0:],
                            start=st, stop=sp,
                        )
                        nc.tensor.matmul(
                            avO[:, col0:], vso[:, kt, pp, :], ex[:, 1, col0:],
                            start=st, stop=sp,
                        )

                    for kt in range(n_kt + 1):
                        if kt < n_kt:
                            emit_sc(kt)
                        if kt > 0:
                            emit_av(kt - 1)
                        yield
                    # denominators: even head at avE row 64, odd at avO row 0.
                    # reciprocal the raw rows on DVE, DMA-broadcast via DRAM,
                    # then the two normalize multiplies.
                    rec = small.tile([128, TCH], F32, tag="rec", name=f"rc{j}{pp}")
                    nc.vector.reciprocal(rec[64:65, :], avE[64:65, :])
                    nc.vector.reciprocal(rec[0:1, :], avO[0:1, :])
                    nc.sync.dma_start(rscr[j, pp, 0:1, :], rec[64:65, :])
                    nc.sync.dma_start(rscr[j, pp, 1:2, :], rec[0:1, :])
                    bc = small.tile([128, TCH], F32, tag="bc", name=f"bc{j}{pp}")
                    nc.sync.dma_start(
                        bc[0:64, :],
                        rscr[j, pp, 0, :][None, :].to_broadcast((64, TCH)),
                    )
                    nc.sync.dma_start(
                        bc[64:128, :],
                        rscr[j, pp, 1, :][None, :].to_broadcast((64, TCH)),
                    )
                    nc.vector.tensor_tensor(
                        ytsb[0:64, pp, ts(j, TCH)], avE[0:64, :], bc[0:64, :],
                        ALU.mult,
                    )
                    nc.vector.tensor_tensor(
                        ytsb[64:128, pp, ts(j, TCH)], avO[64:128, :], bc[64:128, :],
                        ALU.mult,
                    )
                    yield

            def gen_C(j):
                """Output projection for chunk j. Yields between tiles."""
                for i in range(4):
                    qt_i = j * 4 + i
                    for co in range(2):
                        pj = psP.tile([128, TCH], F32, tag="mm", name=f"pj{j}{i}{co}")
                        for cc in range(2):
                            nc.tensor.matmul(
                                pj[:],
                                ytsb[:, cc, ts(qt_i, 128)],
                                wpj_sb[:, cc, ts(co, TCH)],
                                start=(cc == 0),
                                stop=(cc == 1),
                            )
                        ot = outst.tile([128, TCH], F32, tag="ot", name=f"ot{j}{i}{co}")
                        if co == 0:
                            nc.vector.tensor_copy(ot[:], pj[:])
                        else:
                            nc.scalar.copy(ot[:], pj[:])
                        nc.sync.dma_start(y[ts(qt_i, 128), ts(co, TCH)], ot[:])
                        yield

            def drain(g):
                for _ in g:
                    pass

            # software-pipelined emission across the chunk sequence.
            chunks = [(r, j) for r in range(reps) for j in range(NJ)]
            a0 = gen_A(chunks[0][1])
            next(a0)  # xt chunk-0 DMAs
            nc.sync.dma_start(cos_sb[:, ts(0, TCH)], cosp[:, ts(0, TCH)])
            nc.sync.dma_start(sin_sb[:, ts(0, TCH)], sinp[:, ts(0, TCH)])
            for _ in range(12):  # the four Q/K pack groups
                next(a0)
            for _cc in range(8):
                nc.sync.dma_start(wv_sb[:, _cc, :], wv_r[:, _cc, :])
            for _c in range(1, NJ):
                nc.sync.dma_start(cos_sb[:, ts(_c, TCH)], cosp[:, ts(_c, TCH)])
                nc.sync.dma_start(sin_sb[:, ts(_c, TCH)], sinp[:, ts(_c, TCH)])
            nc.sync.dma_start(mask_sb[:], maskt[:])
            nc.sync.dma_start(mask01_sb[:], mask01t[:])
            nc.sync.dma_start(ident_sb[:], identt[:])
            for _cc in range(2):
                nc.sync.dma_start(wpj_sb[:, _cc, :], wpj_r[:, _cc, :])
            drain(a0)

            # deadline-paced weaving: filler generators (projections for the
            # next chunk, output projection for finished chunks) are drained
            # at a per-block rate that spreads each one evenly to its
            # deadline, so thin-filler chunks aren't starved and chunk ends
            # don't dump PE-work bursts while ACT idles.
            class Fill:
                def __init__(self, gen, steps, deadline):
                    self.gen = gen
                    self.rem = steps
                    self.deadline = deadline  # chunk idx it must finish in
                    self.credit = 0.0

                def step(self):
                    try:
                        next(self.gen)
                    except StopIteration:
                        self.rem = 0
                        return
                    self.rem -= 1

            def blocks_of(idx):
                return 2 * (4 * chunks[idx][1] + 6)

            fills = []
            for idx, (r, j) in enumerate(chunks):
                if idx + 1 < len(chunks):
                    fills.append(Fill(gen_A(chunks[idx + 1][1]), 21, idx))
                n_blocks = blocks_of(idx)
                pair_blocks = n_blocks // 2
                done = 0
                for _ in gen_B(j):
                    done += 1
                    w = 1.0
                    for f in sorted(fills, key=lambda f: f.deadline):
                        if f.rem <= 0:
                            continue
                        togo = n_blocks - done
                        if f.deadline > idx and idx + 1 < len(chunks):
                            togo += blocks_of(idx + 1)
                        f.credit += w * f.rem / max(togo, 1)
                        while f.credit >= 1.0 and f.rem > 0:
                            f.step()
                            f.credit -= 1.0
                for f in fills:
                    if f.deadline <= idx:
                        while f.rem > 0:
                            f.step()
                        drain(f.gen)
                fills = [f for f in fills if f.rem > 0]
                lastc = idx + 1 if idx + 1 < len(chunks) else idx
                fills.append(Fill(gen_C(j), 8, lastc))
            for f in fills:
                drain(f.gen)

    nc.compile()
    return nc


@lru_cache(maxsize=8)
def _get_nc(reps=1, variant=""):
    return _build_nc(reps, variant)


def _host_tables():
    dh = DH
    invf = 1.0 / (ROPE_BASE ** (np.arange(0, dh, 2, dtype=np.float64) / dh))
    t = np.arange(T, dtype=np.float64)
    freqs = np.outer(t, invf)  # [T, 32]
    cos_td = np.repeat(np.cos(freqs), 2, axis=1)  # [T, 64]
    sin_td = np.repeat(np.sin(freqs), 2, axis=1)
    cosp = np.tile(cos_td.T, (2, 1)).astype(np.float32)  # [128, T]
    sinp = np.tile(sin_td.T, (2, 1)).astype(np.float32)

    r = np.zeros((64, 64), dtype=np.float32)
    for i in range(32):
        r[2 * i, 2 * i + 1] = -1.0
        r[2 * i + 1, 2 * i] = 1.0
    r2 = np.zeros((128, 128), dtype=np.float32)
    r2[0:64, 0:64] = r
    r2[64:128, 64:128] = r
    r2t = np.ascontiguousarray(r2.T)

    # mt[c, k] = -MASKB where k > c (strictly above the block diagonal)
    rr = np.arange(128)[:, None]  # c
    cc = np.arange(128)[None, :]  # k
    maskt = np.where(cc > rr, -MASKB, 0.0).astype(np.float32)
    mask01t = (rr <= cc).astype(np.float32)
    identt = np.eye(128, dtype=np.float32)
    return cosp, sinp, r2t, maskt, mask01t, identt


def _in_maps(x, Wqkv, Wproj):
    import ml_dtypes

    cosp, sinp, r2t, maskt, mask01t, identt = _host_tables()
    maps = []
    for c in range(NCORES):
        b, g = divmod(c, 4)
        xTc = np.ascontiguousarray(x[b].T).astype(np.float32)
        wq = Wqkv[:, g * 256:(g + 1) * 256]
        wk = Wqkv[:, C + g * 256: C + (g + 1) * 256]
        wvv = Wqkv[:, 2 * C + g * 256: 2 * C + (g + 1) * 256]
        wqkc = np.ascontiguousarray(
            np.concatenate([wq, wk], axis=1), dtype=np.float32
        )
        maps.append(
            {
                "xT": xTc,
                "wqk": wqkc,
                "wv": np.ascontiguousarray(wvv, dtype=np.float32),
                "wpj": np.ascontiguousarray(
                    Wproj[g * 256:(g + 1) * 256, :], dtype=np.float32
                ),
                "r2t": r2t.astype(ml_dtypes.bfloat16),
                "cosp": cosp,
                "sinp": sinp,
                "maskt": maskt.astype(ml_dtypes.bfloat16),
                "mask01t": mask01t.astype(ml_dtypes.bfloat16),
                "identt": identt.astype(ml_dtypes.bfloat16),
            }
        )
    return maps


def _assemble(results):
    out = np.zeros((B, T, C), dtype=np.float32)
    for c in range(NCORES):
        b = c // 4
        out[b] += results[c]["y"]
    return out


def kernel(x, Wqkv, Wproj):
    x = np.asarray(x, dtype=np.float32)
    Wqkv = np.asarray(Wqkv, dtype=np.float32)
    Wproj = np.asarray(Wproj, dtype=np.float32)
    nc = _get_nc()
    maps = _in_maps(x, Wqkv, Wproj)
    res = run_bass_kernel_spmd(nc, maps, core_ids=list(range(NCORES)))
    return _assemble(res.results)


# revision 4
# speedup vs baseline: 44.8016x; 44.8016x over previous
"""Causal self-attention with RoPE on 8 trn2 NeuronCores — v2.

Full inputs -> full output. Sharding: data-parallel on batch (2) x
tensor-parallel on heads (4 heads/core). Each core computes qkv projections,
RoPE, causal attention for its 4 heads, and a partial output projection
(row-parallel slice of Wproj); the host sums the 4 partials per batch.

v2 changes vs v1 (all aimed at PE idle-gap removal; PE is the roofline):
  - causal diagonal masking folded into the scores PSUM accumulation as a
    constant bias matmul (mt^T @ I adds -2^17.58 above the diagonal), so
    exp() output feeds the AV matmul directly: no GPSIMD mask op, no
    Pool->PE dependency chain.
  - RoPE runs in-place in one PSUM bank per pack: proj matmuls accumulate
    ps; DVE writes tsin = ps*sin (SBUF); DVE overwrites ps *= cos; the PE
    rotation matmul accumulates R@tsin onto ps (start=False); one DVE copy
    emits the bf16 pack. Frees 2 PSUM banks vs the v1 rp scheme and
    decouples Q/K-proj PSUM recycling from V/output-proj tiles.
  - PSUM pools: ps (2 banks) / sc (4) / avE+avO (2), with pv/pj sharing
    the ps pool's rotation. No pool oversubscription.
  - the scores stream runs one kt block ahead of the AV matmuls, so each
    exp() executes in the shadow of the next block's scores.
  - filler weaving is deadline-paced globally: next-chunk projections and
    previous-chunk output projections drain at an even per-block rate to
    their deadlines instead of per-chunk lump sums, so thin-filler chunks
    (late j) aren't starved and chunk boundaries don't burst.
  - PSUM-side copies are routed explicitly and balanced across ACT/DVE
    (vse copies + half the output-tile copies on ACT, the rest on DVE):
    'any' routing let copies clump on ACT and stall the sc-bank recycle.
  - softmax skips max-subtraction (scores are ~N(0,1); exp stays well
    inside bf16 range) and gets the denominator from an appended
    ones-column in V.
"""

import math
from functools import lru_cache

import numpy as np

import concourse.bacc as bacc
import concourse.bass as bass
import concourse.mybir as mybir
import concourse.tile as tile
from concourse.bass import ds, ts
from concourse.bass_utils import run_bass_kernel_spmd

B, T, C = 2, 2048, 1024
H, DH = 16, 64
HL = 4  # heads per core
NCORES = 8
ROPE_BASE = 10000.0
MASKB = 196608.0  # causal bias; exactly representable in bf16; /8 -> -24576

F32 = mybir.dt.float32
F32R = mybir.dt.float32r
BF16 = mybir.dt.bfloat16
AF = mybir.ActivationFunctionType
ALU = mybir.AluOpType

TCH = 512  # T chunk (free dim of projection / q chunk of attention)
NJ = T // TCH  # 4
NKT = T // 128  # 16 k tiles


def _build_nc(reps=1, variant=""):
    nc = bacc.Bacc("TRN2", target_bir_lowering=False, debug=False, num_devices=NCORES)

    xT = nc.dram_tensor("xT", [C, T], F32R, kind="ExternalInput")
    wqk = nc.dram_tensor("wqk", [C, 512], F32R, kind="ExternalInput")
    wv = nc.dram_tensor("wv", [C, 256], F32R, kind="ExternalInput")
    wpj = nc.dram_tensor("wpj", [256, C], F32R, kind="ExternalInput")
    r2t = nc.dram_tensor("r2t", [128, 128], BF16, kind="ExternalInput")
    cosp = nc.dram_tensor("cosp", [128, T], F32, kind="ExternalInput")
    sinp = nc.dram_tensor("sinp", [128, T], F32, kind="ExternalInput")
    maskt = nc.dram_tensor("maskt", [128, 128], BF16, kind="ExternalInput")
    mask01t = nc.dram_tensor("mask01t", [128, 128], BF16, kind="ExternalInput")
    identt = nc.dram_tensor("identt", [128, 128], BF16, kind="ExternalInput")
    y = nc.dram_tensor("y", [T, C], F32, kind="ExternalOutput")
    # scratch rows for the denominator partition-broadcast round-trip
    rscr = nc.dram_tensor("rscr", [NJ, 2, 2, TCH], F32, kind="Internal")

    xT_r = xT.rearrange("(co p) t -> p co t", p=128)
    wqk_r = wqk.rearrange("(co p) m -> p co m", p=128)
    wv_r = wv.rearrange("(co p) m -> p co m", p=128)
    wpj_r = wpj.rearrange("(cc p) n -> p cc n", p=128)

    with tile.TileContext(nc) as tc:
        with (
            tc.tile_pool(name="const", bufs=1) as const,
            tc.tile_pool(name="persist", bufs=1) as persist,
            tc.tile_pool(name="work", bufs=3) as work,
            tc.tile_pool(name="expool", bufs=12) as expool,
            tc.tile_pool(name="small", bufs=3) as small,
            tc.tile_pool(name="outst", bufs=8) as outst,
            tc.tile_pool(name="psP", bufs=2, space="PSUM") as psP,
            tc.tile_pool(name="psS", bufs=2, space="PSUM") as psS,
            tc.tile_pool(name="psV", bufs=1, space="PSUM") as psV,
        ):
            # ---- constants ----
            wqk_sb = const.tile([128, 8, 512], F32R)
            wv_sb = const.tile([128, 8, 256], F32R)
            wpj_sb = const.tile([128, 2, 1024], F32R)
            r2t_sb = const.tile([128, 128], BF16)
            mask_sb = const.tile([128, 128], BF16)
            mask01_sb = const.tile([128, 128], BF16)
            ident_sb = const.tile([128, 128], BF16)
            cos_sb = const.tile([128, T], F32)
            sin_sb = const.tile([128, T], F32)
            # first-wave DMAs: only what the first matmuls touch
            for _cc in range(8):
                nc.sync.dma_start(wqk_sb[:, _cc, :], wqk_r[:, _cc, :])
            nc.sync.dma_start(r2t_sb[:], r2t[:])

            # ---- persistent activations ----
            qt_packs = [
                persist.tile([128, T], BF16, tag=f"qt{p}", name=f"qt{p}")
                for p in range(2)
            ]
            kt_packs = [
                persist.tile([128, T], BF16, tag=f"kt{p}", name=f"kt{p}")
                for p in range(2)
            ]
            # V for even local heads: [.., 65] = [v dims | ones]
            vse = persist.tile([128, NKT, 2, 65], BF16, tag="vse")
            # V for odd local heads: [.., 128] = [ones | zeros x63 | v dims]
            vso = persist.tile([128, NKT, 2, 128], BF16, tag="vso")
            ytsb = persist.tile([128, 2, T], F32R, tag="ytsb")

            nc.gpsimd.memset(vse[:, :, :, 64], 1.0)
            nc.gpsimd.memset(vso[:, :, :, 0], 1.0)
            nc.gpsimd.memset(vso[:, :, :, 1:64], 0.0)

            def gen_A(j):
                """Projections + RoPE + V for chunk j. Yields between groups."""
                xt = work.tile([128, 8, TCH], F32R, tag="xt", name=f"xt{j}")
                for _cc in range(8):
                    nc.sync.dma_start(xt[:, _cc, :], xT_r[:, _cc, ts(j, TCH)])
                yield
                for pk in range(4):  # Q pack0, Q pack1, K pack0, K pack1
                    ps = psP.tile([128, TCH], F32, tag="mm", name=f"ps{j}{pk}")
                    for cc in range(4):
                        nc.tensor.matmul(
                            ps[:],
                            wqk_sb[:, cc, ts(pk, 128)],
                            xt[:, cc, :],
                            start=(cc == 0),
                            stop=False,
                        )
                    yield
                    for cc in range(4, 8):
                        nc.tensor.matmul(
                            ps[:],
                            wqk_sb[:, cc, ts(pk, 128)],
                            xt[:, cc, :],
                            start=False,
                            stop=(cc == 7),
                        )
                    yield
                    # rot(q*sin) == rot(q)*sin: sin-multiply straight off PSUM;
                    # the three DVE ops are emitted back-to-back so no other
                    # DVE work lands inside the ps-recycle chain
                    tsin = work.tile([128, TCH], BF16, tag="tsin", name=f"tsn{j}{pk}")
                    nc.vector.tensor_tensor(
                        tsin[:], ps[:], sin_sb[:, ts(j, TCH)], ALU.mult
                    )
                    # in-place: ps *= cos, then accumulate R @ tsin onto it
                    nc.vector.tensor_tensor(
                        ps[:], ps[:], cos_sb[:, ts(j, TCH)], ALU.mult
                    )
                    nc.tensor.matmul(
                        ps[:], r2t_sb[:], tsin[:],
                        start=False, stop=True, skip_group_check=True,
                    )
                    dest = (qt_packs + kt_packs)[pk]
                    nc.vector.tensor_copy(dest[:, ts(j, TCH)], ps[:])
                    yield
                for i in range(4):  # V t-tiles of this chunk
                    kt_i = j * 4 + i
                    pv = psP.tile([128, 256], F32, tag="mm", name=f"pv{j}{i}")
                    for cc in range(4):
                        nc.tensor.matmul(
                            pv[:],
                            xt[:, cc, ts(i, 128)],
                            wv_sb[:, cc, :],
                            start=(cc == 0),
                            stop=False,
                        )
                    yield
                    for cc in range(4, 8):
                        nc.tensor.matmul(
                            pv[:],
                            xt[:, cc, ts(i, 128)],
                            wv_sb[:, cc, :],
                            start=False,
                            stop=(cc == 7),
                        )
                    pv4 = pv[:].rearrange("p (h d) -> p h d", h=4)
                    nc.scalar.copy(vse[:, kt_i, :, 0:64], pv4[:, 0::2, :])
                    nc.vector.tensor_copy(vso[:, kt_i, :, 64:128], pv4[:, 1::2, :])
                    yield

            def gen_B(j):
                """Attention for q chunk j. Yields between kt blocks.

                Scores run one block ahead of the AV matmuls so each exp()
                executes in the shadow of the next block's scores.
                """
                for pp in range(2):  # head pairs (2pp, 2pp+1)
                    KT = kt_packs[pp]
                    QT = qt_packs[pp]
                    n_kt = 4 * j + 4
                    avE = psV.tile([65, TCH], F32, tag="avE", name=f"avE{j}{pp}")
                    avO = psV.tile([128, TCH], F32, tag="avO", name=f"avO{j}{pp}")
                    exs = {}

                    def emit_sc(kt):
                        d = kt - 4 * j
                        col0 = max(d, 0) * 128
                        sc = psS.tile(
                            [128, 2, TCH], F32, tag="sc", name=f"sc{j}{pp}{kt}"
                        )
                        if d >= 0 and "poolmask" in variant:
                            qsl = ds(j * TCH + col0, TCH - col0)
                            nc.tensor.matmul(
                                sc[:, 0, col0:], KT[0:64, ts(kt, 128)],
                                QT[0:64, qsl],
                                start=True, stop=True, tile_position=(0, 0),
                            )
                            nc.tensor.matmul(
                                sc[:, 1, col0:], KT[64:128, ts(kt, 128)],
                                QT[64:128, qsl],
                                start=True, stop=True, tile_position=(64, 0),
                            )
                        elif d >= 0:
                            # causal bias on the diagonal 128-block, then the
                            # two 64-contract head matmuls accumulate onto it
                            asl = slice(col0, col0 + 128)
                            qasl = ds(j * TCH + col0, 128)
                            for h in range(2):
                                nc.tensor.matmul(
                                    sc[:, h, asl],
                                    mask_sb[:],
                                    ident_sb[:],
                                    start=True, stop=False,
                                    skip_group_check=True,
                                )
                                nc.tensor.matmul(
                                    sc[:, h, asl],
                                    KT[64 * h:64 * h + 64, ts(kt, 128)],
                                    QT[64 * h:64 * h + 64, qasl],
                                    start=False, stop=True,
                                    skip_group_check=True,
                                    tile_position=(64 * h, 0),
                                )
                            if col0 + 128 < TCH:
                                bsl = ds(j * TCH + col0 + 128, TCH - col0 - 128)
                                nc.tensor.matmul(
                                    sc[:, 0, col0 + 128:],
                                    KT[0:64, ts(kt, 128)], QT[0:64, bsl],
                                    start=True, stop=True, tile_position=(0, 0),
                                )
                                nc.tensor.matmul(
                                    sc[:, 1, col0 + 128:],
                                    KT[64:128, ts(kt, 128)], QT[64:128, bsl],
                                    start=True, stop=True, tile_position=(64, 0),
                                )
                        else:
                            qsl = ds(j * TCH, TCH)
                            nc.tensor.matmul(
                                sc[:, 0, :], KT[0:64, ts(kt, 128)], QT[0:64, qsl],
                                start=True, stop=True, tile_position=(0, 0),
                            )
                            nc.tensor.matmul(
                                sc[:, 1, :], KT[64:128, ts(kt, 128)], QT[64:128, qsl],
                                start=True, stop=True, tile_position=(64, 0),
                            )
                        ex = expool.tile(
                            [128, 2, TCH], BF16, tag="ex", name=f"ex{j}{pp}{kt}"
                        )
                        if "noexp" not in variant:
                            nc.scalar.activation(
                                ex[:, :, col0:], sc[:, :, col0:], AF.Exp,
                                scale=1.0 / math.sqrt(DH),
                            )
                        if d >= 0 and "poolmask" in variant:
                            msl = slice(col0, col0 + 128)
                            nc.gpsimd.tensor_tensor(
                                ex[:, :, msl],
                                ex[:, :, msl],
                                mask01_sb[:, None, :].to_broadcast((128, 2, 128)),
                                ALU.mult,
                            )
                        exs[kt] = (ex, col0)

                    def emit_av(kt):
                        ex, col0 = exs.pop(kt)
                        st = kt == 0
                        sp = kt == n_kt - 1
                        nc.tensor.matmul(
                            avE[:, col0:], vse[:, kt, pp, :], ex[:, 0, col0:],
                            start=st, stop=sp,
                        )
                        nc.tensor.matmul(
                            avO[:, col0:], vso[:, kt, pp, :], ex[:, 1, col0:],
                            start=st, stop=sp,
                        )

                    for kt in range(n_kt + 1):
                        if kt < n_kt:
                            emit_sc(kt)
                        if kt > 0:
                            emit_av(kt - 1)
                        yield
                    # denominators: even head at avE row 64, odd at avO row 0.
                    # reciprocal the raw rows on DVE, DMA-broadcast via DRAM,
                    # then the two normalize multiplies.
                    rec = small.tile([128, TCH], F32, tag="rec", name=f"rc{j}{pp}")
                    nc.vector.reciprocal(rec[64:65, :], avE[64:65, :])
                    nc.vector.reciprocal(rec[0:1, :], avO[0:1, :])
                    nc.sync.dma_start(rscr[j, pp, 0:1, :], rec[64:65, :])
                    nc.sync.dma_start(rscr[j, pp, 1:2, :], rec[0:1, :])
                    bc = small.tile([128, TCH], F32, tag="bc", name=f"bc{j}{pp}")
                    nc.sync.dma_start(
                        bc[0:64, :],
                        rscr[j, pp, 0, :][None, :].to_broadcast((64, TCH)),
                    )
                    nc.sync.dma_start(
                        bc[64:128, :],
                        rscr[j, pp, 1, :][None, :].to_broadcast((64, TCH)),
                    )
                    nc.vector.tensor_tensor(
                        ytsb[0:64, pp, ts(j, TCH)], avE[0:64, :], bc[0:64, :],
                        ALU.mult,
                    )
                    nc.vector.tensor_tensor(
                        ytsb[64:128, pp, ts(j, TCH)], avO[64:128, :], bc[64:128, :],
                        ALU.mult,
                    )
                    yield

            def gen_C(j):
                """Output projection for chunk j. Yields between tiles."""
                for i in range(4):
                    qt_i = j * 4 + i
                    for co in range(2):
                        pj = psP.tile([128, TCH], F32, tag="mm", name=f"pj{j}{i}{co}")
                        for cc in range(2):
                            nc.tensor.matmul(
                                pj[:],
                                ytsb[:, cc, ts(qt_i, 128)],
                                wpj_sb[:, cc, ts(co, TCH)],
                                start=(cc == 0),
                                stop=(cc == 1),
                            )
                        ot = outst.tile([128, TCH], F32, tag="ot", name=f"ot{j}{i}{co}")
                        if co == 0:
                            nc.vector.tensor_copy(ot[:], pj[:])
                        else:
                            nc.scalar.copy(ot[:], pj[:])
                        nc.sync.dma_start(y[ts(qt_i, 128), ts(co, TCH)], ot[:])
                        yield

            def drain(g):
                for _ in g:
                    pass

            # software-pipelined emission across the chunk sequence.
            chunks = [(r, j) for r in range(reps) for j in range(NJ)]
            a0 = gen_A(chunks[0][1])
            next(a0)  # xt chunk-0 DMAs
            nc.sync.dma_start(cos_sb[:, ts(0, TCH)], cosp[:, ts(0, TCH)])
            nc.sync.dma_start(sin_sb[:, ts(0, TCH)], sinp[:, ts(0, TCH)])
            for _ in range(12):  # the four Q/K pack groups
                next(a0)
            for _cc in range(8):
                nc.sync.dma_start(wv_sb[:, _cc, :], wv_r[:, _cc, :])
            for _c in range(1, NJ):
                nc.sync.dma_start(cos_sb[:, ts(_c, TCH)], cosp[:, ts(_c, TCH)])
                nc.sync.dma_start(sin_sb[:, ts(_c, TCH)], sinp[:, ts(_c, TCH)])
            nc.sync.dma_start(mask_sb[:], maskt[:])
            nc.sync.dma_start(mask01_sb[:], mask01t[:])
            nc.sync.dma_start(ident_sb[:], identt[:])
            for _cc in range(2):
                nc.sync.dma_start(wpj_sb[:, _cc, :], wpj_r[:, _cc, :])
            drain(a0)

            # deadline-paced weaving: filler generators (projections for the
            # next chunk, output projection for finished chunks) are drained
            # at a per-block rate that spreads each one evenly to its
            # deadline, so thin-filler chunks aren't starved and chunk ends
            # don't dump PE-work bursts while ACT idles.
            class Fill:
                def __init__(self, gen, steps, deadline):
                    self.gen = gen
                    self.rem = steps
                    self.deadline = deadline  # chunk idx it must finish in
                    self.credit = 0.0

                def step(self):
                    try:
                        next(self.gen)
                    except StopIteration:
                        self.rem = 0
                        return
                    self.rem -= 1

            def blocks_of(idx):
                return 2 * (4 * chunks[idx][1] + 6)

            fills = []
            for idx, (r, j) in enumerate(chunks):
                if idx + 1 < len(chunks):
                    fills.append(Fill(gen_A(chunks[idx + 1][1]), 21, idx))
                n_blocks = blocks_of(idx)
                pair_blocks = n_blocks // 2
                done = 0
                for _ in gen_B(j):
                    done += 1
                    w = 1.0
                    for f in sorted(fills, key=lambda f: f.deadline):
                        if f.rem <= 0:
                            continue
                        togo = n_blocks - done
                        if f.deadline > idx and idx + 1 < len(chunks):
                            togo += blocks_of(idx + 1)
                        f.credit += w * f.rem / max(togo, 1)
                        while f.credit >= 1.0 and f.rem > 0:
                            f.step()
                            f.credit -= 1.0
                for f in fills:
                    if f.deadline <= idx:
                        while f.rem > 0:
                            f.step()
                        drain(f.gen)
                fills = [f for f in fills if f.rem > 0]
                lastc = idx + 1 if idx + 1 < len(chunks) else idx
                fills.append(Fill(gen_C(j), 8, lastc))
            for f in fills:
                drain(f.gen)

    nc.compile()
    return nc


@lru_cache(maxsize=8)
def _get_nc(reps=1, variant=""):
    return _build_nc(reps, variant)


def _host_tables():
    dh = DH
    invf = 1.0 / (ROPE_BASE ** (np.arange(0, dh, 2, dtype=np.float64) / dh))
    t = np.arange(T, dtype=np.float64)
    freqs = np.outer(t, invf)  # [T, 32]
    cos_td = np.repeat(np.cos(freqs), 2, axis=1)  # [T, 64]
    sin_td = np.repeat(np.sin(freqs), 2, axis=1)
    cosp = np.tile(cos_td.T, (2, 1)).astype(np.float32)  # [128, T]
    sinp = np.tile(sin_td.T, (2, 1)).astype(np.float32)

    r = np.zeros((64, 64), dtype=np.float32)
    for i in range(32):
        r[2 * i, 2 * i + 1] = -1.0
        r[2 * i + 1, 2 * i] = 1.0
    r2 = np.zeros((128, 128), dtype=np.float32)
    r2[0:64, 0:64] = r
    r2[64:128, 64:128] = r
    r2t = np.ascontiguousarray(r2.T)

    # mt[c, k] = -MASKB where k > c (strictly above the block diagonal)
    rr = np.arange(128)[:, None]  # c
    cc = np.arange(128)[None, :]  # k
    maskt = np.where(cc > rr, -MASKB, 0.0).astype(np.float32)
    mask01t = (rr <= cc).astype(np.float32)
    identt = np.eye(128, dtype=np.float32)
    return cosp, sinp, r2t, maskt, mask01t, identt


def _in_maps(x, Wqkv, Wproj):
    import ml_dtypes

    cosp, sinp, r2t, maskt, mask01t, identt = _host_tables()
    maps = []
    for c in range(NCORES):
        b, g = divmod(c, 4)
        xTc = np.ascontiguousarray(x[b].T).astype(np.float32)
        wq = Wqkv[:, g * 256:(g + 1) * 256]
        wk = Wqkv[:, C + g * 256: C + (g + 1) * 256]
        wvv = Wqkv[:, 2 * C + g * 256: 2 * C + (g + 1) * 256]
        wqkc = np.ascontiguousarray(
            np.concatenate([wq, wk], axis=1), dtype=np.float32
        )
        maps.append(
            {
                "xT": xTc,
                "wqk": wqkc,
                "wv": np.ascontiguousarray(wvv, dtype=np.float32),
                "wpj": np.ascontiguousarray(
                    Wproj[g * 256:(g + 1) * 256, :], dtype=np.float32
                ),
                "r2t": r2t.astype(ml_dtypes.bfloat16),
                "cosp": cosp,
                "sinp": sinp,
                "maskt": maskt.astype(ml_dtypes.bfloat16),
                "mask01t": mask01t.astype(ml_dtypes.bfloat16),
                "identt": identt.astype(ml_dtypes.bfloat16),
            }
        )
    return maps


def _assemble(results):
    out = np.zeros((B, T, C), dtype=np.float32)
    for c in range(NCORES):
        b = c // 4
        out[b] += results[c]["y"]
    return out


def kernel(x, Wqkv, Wproj):
    x = np.asarray(x, dtype=np.float32)
    Wqkv = np.asarray(Wqkv, dtype=np.float32)
    Wproj = np.asarray(Wproj, dtype=np.float32)
    nc = _get_nc()
    maps = _in_maps(x, Wqkv, Wproj)
    res = run_bass_kernel_spmd(nc, maps, core_ids=list(range(NCORES)))
    return _assemble(res.results)


# revision 6
# speedup vs baseline: 46.0255x; 1.0273x over previous
"""Causal self-attention with RoPE on 8 trn2 NeuronCores — v2.

Full inputs -> full output. Sharding: data-parallel on batch (2) x
tensor-parallel on heads (4 heads/core). Each core computes qkv projections,
RoPE, causal attention for its 4 heads, and a partial output projection
(row-parallel slice of Wproj); the host sums the 4 partials per batch.

v2 changes vs v1 (all aimed at PE idle-gap removal; PE is the roofline):
  - causal diagonal masking folded into the scores PSUM accumulation as a
    constant bias matmul (mt^T @ I adds -2^17.58 above the diagonal), so
    exp() output feeds the AV matmul directly: no GPSIMD mask op, no
    Pool->PE dependency chain.
  - RoPE runs in-place in one PSUM bank per pack: proj matmuls accumulate
    ps; DVE writes tsin = ps*sin (SBUF); DVE overwrites ps *= cos; the PE
    rotation matmul accumulates R@tsin onto ps (start=False); one DVE copy
    emits the bf16 pack. Frees 2 PSUM banks vs the v1 rp scheme and
    decouples Q/K-proj PSUM recycling from V/output-proj tiles.
  - PSUM pools: ps (2 banks) / sc (4) / avE+avO (2), with pv/pj sharing
    the ps pool's rotation. No pool oversubscription.
  - the scores stream runs one kt block ahead of the AV matmuls, so each
    exp() executes in the shadow of the next block's scores.
  - filler weaving is deadline-paced globally: next-chunk projections
    finish within the current chunk, while each finished chunk's output
    projection spreads over the next THREE chunks (ytsb[j] isn't rewritten
    until the next pass), so thin-filler chunks aren't starved and chunk
    boundaries don't burst.
  - PSUM-side copies are routed explicitly and balanced across ACT/DVE
    (vse copies + half the output-tile copies on ACT, the rest on DVE):
    'any' routing let copies clump on ACT and stall the sc-bank recycle.
  - softmax skips max-subtraction (scores are ~N(0,1); exp stays well
    inside bf16 range) and gets the denominator from an appended
    ones-column in V.
"""

import math
from functools import lru_cache

import numpy as np

import concourse.bacc as bacc
import concourse.bass as bass
import concourse.mybir as mybir
import concourse.tile as tile
from concourse.bass import ds, ts
from concourse.bass_utils import run_bass_kernel_spmd

B, T, C = 2, 2048, 1024
H, DH = 16, 64
HL = 4  # heads per core
NCORES = 8
ROPE_BASE = 10000.0
MASKB = 196608.0  # causal bias; exactly representable in bf16; /8 -> -24576

F32 = mybir.dt.float32
F32R = mybir.dt.float32r
BF16 = mybir.dt.bfloat16
AF = mybir.ActivationFunctionType
ALU = mybir.AluOpType

TCH = 512  # T chunk (free dim of projection / q chunk of attention)
NJ = T // TCH  # 4
NKT = T // 128  # 16 k tiles


def _build_nc(reps=1, variant=""):
    nc = bacc.Bacc("TRN2", target_bir_lowering=False, debug=False, num_devices=NCORES)

    xT = nc.dram_tensor("xT", [C, T], F32R, kind="ExternalInput")
    wqk = nc.dram_tensor("wqk", [C, 512], F32R, kind="ExternalInput")
    wv = nc.dram_tensor("wv", [C, 256], F32R, kind="ExternalInput")
    wpj = nc.dram_tensor("wpj", [256, C], F32R, kind="ExternalInput")
    r2t = nc.dram_tensor("r2t", [128, 128], BF16, kind="ExternalInput")
    cosp = nc.dram_tensor("cosp", [128, T], F32, kind="ExternalInput")
    sinp = nc.dram_tensor("sinp", [128, T], F32, kind="ExternalInput")
    maskt = nc.dram_tensor("maskt", [128, 128], BF16, kind="ExternalInput")
    mask01t = nc.dram_tensor("mask01t", [128, 128], BF16, kind="ExternalInput")
    identt = nc.dram_tensor("identt", [128, 128], BF16, kind="ExternalInput")
    y = nc.dram_tensor("y", [T, C], F32, kind="ExternalOutput")
    # scratch rows for the denominator partition-broadcast round-trip
    rscr = nc.dram_tensor("rscr", [NJ, 2, 2, TCH], F32, kind="Internal")

    xT_r = xT.rearrange("(co p) t -> p co t", p=128)
    wqk_r = wqk.rearrange("(co p) m -> p co m", p=128)
    wv_r = wv.rearrange("(co p) m -> p co m", p=128)
    wpj_r = wpj.rearrange("(cc p) n -> p cc n", p=128)

    with tile.TileContext(nc) as tc:
        with (
            tc.tile_pool(name="const", bufs=1) as const,
            tc.tile_pool(name="persist", bufs=1) as persist,
            tc.tile_pool(name="work", bufs=3) as work,
            tc.tile_pool(name="expool", bufs=12) as expool,
            tc.tile_pool(name="small", bufs=3) as small,
            tc.tile_pool(name="outst", bufs=8) as outst,
            tc.tile_pool(name="psP", bufs=2, space="PSUM") as psP,
            tc.tile_pool(name="psS", bufs=2, space="PSUM") as psS,
            tc.tile_pool(name="psV", bufs=1, space="PSUM") as psV,
        ):
            # ---- constants ----
            wqk_sb = const.tile([128, 8, 512], F32R)
            wv_sb = const.tile([128, 8, 256], F32R)
            wpj_sb = const.tile([128, 2, 1024], F32R)
            r2t_sb = const.tile([128, 128], BF16)
            mask_sb = const.tile([128, 128], BF16)
            mask01_sb = const.tile([128, 128], BF16)
            ident_sb = const.tile([128, 128], BF16)
            cos_sb = const.tile([128, T], F32)
            sin_sb = const.tile([128, T], F32)
            # first-wave DMAs: only what the first matmuls touch
            for _cc in range(8):
                nc.sync.dma_start(wqk_sb[:, _cc, :], wqk_r[:, _cc, :])
            nc.sync.dma_start(r2t_sb[:], r2t[:])

            # ---- persistent activations ----
            qt_packs = [
                persist.tile([128, T], BF16, tag=f"qt{p}", name=f"qt{p}")
                for p in range(2)
            ]
            kt_packs = [
                persist.tile([128, T], BF16, tag=f"kt{p}", name=f"kt{p}")
                for p in range(2)
            ]
            # V for even local heads: [.., 65] = [v dims | ones]
            vse = persist.tile([128, NKT, 2, 65], BF16, tag="vse")
            # V for odd local heads: [.., 128] = [ones | zeros x63 | v dims]
            vso = persist.tile([128, NKT, 2, 128], BF16, tag="vso")
            ytsb = persist.tile([128, 2, T], F32R, tag="ytsb")

            nc.gpsimd.memset(vse[:, :, :, 64], 1.0)
            nc.gpsimd.memset(vso[:, :, :, 0], 1.0)
            nc.gpsimd.memset(vso[:, :, :, 1:64], 0.0)

            def gen_A(j):
                """Projections + RoPE + V for chunk j. Yields between groups."""
                xt = work.tile([128, 8, TCH], F32R, tag="xt", name=f"xt{j}")
                for _cc in range(8):
                    nc.sync.dma_start(xt[:, _cc, :], xT_r[:, _cc, ts(j, TCH)])
                yield
                for pk in range(4):  # Q pack0, Q pack1, K pack0, K pack1
                    ps = psP.tile([128, TCH], F32, tag="mm", name=f"ps{j}{pk}")
                    for cc in range(4):
                        nc.tensor.matmul(
                            ps[:],
                            wqk_sb[:, cc, ts(pk, 128)],
                            xt[:, cc, :],
                            start=(cc == 0),
                            stop=False,
                        )
                    yield
                    for cc in range(4, 8):
                        nc.tensor.matmul(
                            ps[:],
                            wqk_sb[:, cc, ts(pk, 128)],
                            xt[:, cc, :],
                            start=False,
                            stop=(cc == 7),
                        )
                    yield
                    # rot(q*sin) == rot(q)*sin: sin-multiply straight off PSUM;
                    # the three DVE ops are emitted back-to-back so no other
                    # DVE work lands inside the ps-recycle chain
                    tsin = work.tile([128, TCH], BF16, tag="tsin", name=f"tsn{j}{pk}")
                    nc.vector.tensor_tensor(
                        tsin[:], ps[:], sin_sb[:, ts(j, TCH)], ALU.mult
                    )
                    # in-place: ps *= cos, then accumulate R @ tsin onto it
                    nc.vector.tensor_tensor(
                        ps[:], ps[:], cos_sb[:, ts(j, TCH)], ALU.mult
                    )
                    nc.tensor.matmul(
                        ps[:], r2t_sb[:], tsin[:],
                        start=False, stop=True, skip_group_check=True,
                    )
                    dest = (qt_packs + kt_packs)[pk]
                    nc.vector.tensor_copy(dest[:, ts(j, TCH)], ps[:])
                    yield
                for i in range(4):  # V t-tiles of this chunk
                    kt_i = j * 4 + i
                    pv = psP.tile([128, 256], F32, tag="mm", name=f"pv{j}{i}")
                    for cc in range(4):
                        nc.tensor.matmul(
                            pv[:],
                            xt[:, cc, ts(i, 128)],
                            wv_sb[:, cc, :],
                            start=(cc == 0),
                            stop=False,
                        )
                    yield
                    for cc in range(4, 8):
                        nc.tensor.matmul(
                            pv[:],
                            xt[:, cc, ts(i, 128)],
                            wv_sb[:, cc, :],
                            start=False,
                            stop=(cc == 7),
                        )
                    pv4 = pv[:].rearrange("p (h d) -> p h d", h=4)
                    nc.scalar.copy(vse[:, kt_i, :, 0:64], pv4[:, 0::2, :])
                    nc.vector.tensor_copy(vso[:, kt_i, :, 64:128], pv4[:, 1::2, :])
                    yield

            def gen_B(j):
                """Attention for q chunk j. Yields between kt blocks.

                Scores run one block ahead of the AV matmuls so each exp()
                executes in the shadow of the next block's scores.
                """
                for pp in range(2):  # head pairs (2pp, 2pp+1)
                    KT = kt_packs[pp]
                    QT = qt_packs[pp]
                    n_kt = 4 * j + 4
                    avE = psV.tile([65, TCH], F32, tag="avE", name=f"avE{j}{pp}")
                    avO = psV.tile([128, TCH], F32, tag="avO", name=f"avO{j}{pp}")
                    exs = {}

                    def emit_sc(kt):
                        d = kt - 4 * j
                        col0 = max(d, 0) * 128
                        sc = psS.tile(
                            [128, 2, TCH], F32, tag="sc", name=f"sc{j}{pp}{kt}"
                        )
                        if d >= 0 and "poolmask" in variant:
                            qsl = ds(j * TCH + col0, TCH - col0)
                            nc.tensor.matmul(
                                sc[:, 0, col0:], KT[0:64, ts(kt, 128)],
                                QT[0:64, qsl],
                                start=True, stop=True, tile_position=(0, 0),
                            )
                            nc.tensor.matmul(
                                sc[:, 1, col0:], KT[64:128, ts(kt, 128)],
                                QT[64:128, qsl],
                                start=True, stop=True, tile_position=(64, 0),
                            )
                        elif d >= 0:
                            # causal bias on the diagonal 128-block, then the
                            # two 64-contract head matmuls accumulate onto it
                            asl = slice(col0, col0 + 128)
                            qasl = ds(j * TCH + col0, 128)
                            for h in range(2):
                                nc.tensor.matmul(
                                    sc[:, h, asl],
                                    mask_sb[:],
                                    ident_sb[:],
                                    start=True, stop=False,
                                    skip_group_check=True,
                                )
                                nc.tensor.matmul(
                                    sc[:, h, asl],
                                    KT[64 * h:64 * h + 64, ts(kt, 128)],
                                    QT[64 * h:64 * h + 64, qasl],
                                    start=False, stop=True,
                                    skip_group_check=True,
                                    tile_position=(64 * h, 0),
                                )
                            if col0 + 128 < TCH:
                                bsl = ds(j * TCH + col0 + 128, TCH - col0 - 128)
                                nc.tensor.matmul(
                                    sc[:, 0, col0 + 128:],
                                    KT[0:64, ts(kt, 128)], QT[0:64, bsl],
                                    start=True, stop=True, tile_position=(0, 0),
                                )
                                nc.tensor.matmul(
                                    sc[:, 1, col0 + 128:],
                                    KT[64:128, ts(kt, 128)], QT[64:128, bsl],
                                    start=True, stop=True, tile_position=(64, 0),
                                )
                        else:
                            qsl = ds(j * TCH, TCH)
                            nc.tensor.matmul(
                                sc[:, 0, :], KT[0:64, ts(kt, 128)], QT[0:64, qsl],
                                start=True, stop=True, tile_position=(0, 0),
                            )
                            nc.tensor.matmul(
                                sc[:, 1, :], KT[64:128, ts(kt, 128)], QT[64:128, qsl],
                                start=True, stop=True, tile_position=(64, 0),
                            )
                        ex = expool.tile(
                            [128, 2, TCH], BF16, tag="ex", name=f"ex{j}{pp}{kt}"
                        )
                        if "noexp" not in variant:
                            nc.scalar.activation(
                                ex[:, :, col0:], sc[:, :, col0:], AF.Exp,
                                scale=1.0 / math.sqrt(DH),
                            )
                        if d >= 0 and "poolmask" in variant:
                            msl = slice(col0, col0 + 128)
                            nc.gpsimd.tensor_tensor(
                                ex[:, :, msl],
                                ex[:, :, msl],
                                mask01_sb[:, None, :].to_broadcast((128, 2, 128)),
                                ALU.mult,
                            )
                        exs[kt] = (ex, col0)

                    def emit_av(kt):
                        ex, col0 = exs.pop(kt)
                        st = kt == 0
                        sp = kt == n_kt - 1
                        nc.tensor.matmul(
                            avE[:, col0:], vse[:, kt, pp, :], ex[:, 0, col0:],
                            start=st, stop=sp,
                        )
                        nc.tensor.matmul(
                            avO[:, col0:], vso[:, kt, pp, :], ex[:, 1, col0:],
                            start=st, stop=sp,
                        )

                    for kt in range(n_kt + 1):
                        if kt < n_kt:
                            emit_sc(kt)
                        if kt > 0:
                            emit_av(kt - 1)
                        yield
                    # denominators: even head at avE row 64, odd at avO row 0.
                    # reciprocal the raw rows on DVE, DMA-broadcast via DRAM,
                    # then the two normalize multiplies.
                    rec = small.tile([128, TCH], F32, tag="rec", name=f"rc{j}{pp}")
                    nc.vector.reciprocal(rec[64:65, :], avE[64:65, :])
                    nc.vector.reciprocal(rec[0:1, :], avO[0:1, :])
                    nc.sync.dma_start(rscr[j, pp, 0:1, :], rec[64:65, :])
                    nc.sync.dma_start(rscr[j, pp, 1:2, :], rec[0:1, :])
                    bc = small.tile([128, TCH], F32, tag="bc", name=f"bc{j}{pp}")
                    nc.sync.dma_start(
                        bc[0:64, :],
                        rscr[j, pp, 0, :][None, :].to_broadcast((64, TCH)),
                    )
                    nc.sync.dma_start(
                        bc[64:128, :],
                        rscr[j, pp, 1, :][None, :].to_broadcast((64, TCH)),
                    )
                    nc.vector.tensor_tensor(
                        ytsb[0:64, pp, ts(j, TCH)], avE[0:64, :], bc[0:64, :],
                        ALU.mult,
                    )
                    nc.vector.tensor_tensor(
                        ytsb[64:128, pp, ts(j, TCH)], avO[64:128, :], bc[64:128, :],
                        ALU.mult,
                    )
                    yield

            def gen_C(j):
                """Output projection for chunk j. Yields between tiles."""
                for i in range(4):
                    qt_i = j * 4 + i
                    for co in range(2):
                        pj = psP.tile([128, TCH], F32, tag="mm", name=f"pj{j}{i}{co}")
                        for cc in range(2):
                            nc.tensor.matmul(
                                pj[:],
                                ytsb[:, cc, ts(qt_i, 128)],
                                wpj_sb[:, cc, ts(co, TCH)],
                                start=(cc == 0),
                                stop=(cc == 1),
                            )
                        ot = outst.tile([128, TCH], F32, tag="ot", name=f"ot{j}{i}{co}")
                        if co == 0:
                            nc.vector.tensor_copy(ot[:], pj[:])
                        else:
                            nc.scalar.copy(ot[:], pj[:])
                        nc.sync.dma_start(y[ts(qt_i, 128), ts(co, TCH)], ot[:])
                        yield

            def drain(g):
                for _ in g:
                    pass

            # software-pipelined emission across the chunk sequence.
            chunks = [(r, j) for r in range(reps) for j in range(NJ)]
            a0 = gen_A(chunks[0][1])
            next(a0)  # xt chunk-0 DMAs
            nc.sync.dma_start(cos_sb[:, ts(0, TCH)], cosp[:, ts(0, TCH)])
            nc.sync.dma_start(sin_sb[:, ts(0, TCH)], sinp[:, ts(0, TCH)])
            for _ in range(12):  # the four Q/K pack groups
                next(a0)
            for _cc in range(8):
                nc.sync.dma_start(wv_sb[:, _cc, :], wv_r[:, _cc, :])
            for _c in range(1, NJ):
                nc.sync.dma_start(cos_sb[:, ts(_c, TCH)], cosp[:, ts(_c, TCH)])
                nc.sync.dma_start(sin_sb[:, ts(_c, TCH)], sinp[:, ts(_c, TCH)])
            nc.sync.dma_start(mask_sb[:], maskt[:])
            nc.sync.dma_start(mask01_sb[:], mask01t[:])
            nc.sync.dma_start(ident_sb[:], identt[:])
            for _cc in range(2):
                nc.sync.dma_start(wpj_sb[:, _cc, :], wpj_r[:, _cc, :])
            drain(a0)

            # deadline-paced weaving: filler generators (projections for the
            # next chunk, output projection for finished chunks) are drained
            # at a per-block rate that spreads each one evenly to its
            # deadline, so thin-filler chunks aren't starved and chunk ends
            # don't dump PE-work bursts while ACT idles.
            class Fill:
                def __init__(self, gen, steps, deadline):
                    self.gen = gen
                    self.rem = steps
                    self.deadline = deadline  # chunk idx it must finish in
                    self.credit = 0.0

                def step(self):
                    try:
                        next(self.gen)
                    except StopIteration:
                        self.rem = 0
                        return
                    self.rem -= 1

            def blocks_of(idx):
                return 2 * (4 * chunks[idx][1] + 6)

            fills = []
            for idx, (r, j) in enumerate(chunks):
                if idx + 1 < len(chunks):
                    fills.append(Fill(gen_A(chunks[idx + 1][1]), 21, idx))
                n_blocks = blocks_of(idx)
                pair_blocks = n_blocks // 2
                done = 0
                for _ in gen_B(j):
                    done += 1
                    w = 1.0
                    for f in sorted(fills, key=lambda f: f.deadline):
                        if f.rem <= 0:
                            continue
                        togo = n_blocks - done
                        for k in range(idx + 1, min(f.deadline, len(chunks) - 1) + 1):
                            togo += blocks_of(k)
                        f.credit += w * f.rem / max(togo, 1)
                        while f.credit >= 1.0 and f.rem > 0:
                            f.step()
                            f.credit -= 1.0
                for f in fills:
                    if f.deadline <= idx:
                        while f.rem > 0:
                            f.step()
                        drain(f.gen)
                fills = [f for f in fills if f.rem > 0]
                lastc = min(idx + 3, len(chunks) - 1)
                fills.append(Fill(gen_C(j), 8, lastc))
            for f in fills:
                drain(f.gen)

    nc.compile()
    return nc


@lru_cache(maxsize=8)
def _get_nc(reps=1, variant=""):
    return _build_nc(reps, variant)


def _host_tables():
    dh = DH
    invf = 1.0 / (ROPE_BASE ** (np.arange(0, dh, 2, dtype=np.float64) / dh))
    t = np.arange(T, dtype=np.float64)
    freqs = np.outer(t, invf)  # [T, 32]
    cos_td = np.repeat(np.cos(freqs), 2, axis=1)  # [T, 64]
    sin_td = np.repeat(np.sin(freqs), 2, axis=1)
    cosp = np.tile(cos_td.T, (2, 1)).astype(np.float32)  # [128, T]
    sinp = np.tile(sin_td.T, (2, 1)).astype(np.float32)

    r = np.zeros((64, 64), dtype=np.float32)
    for i in range(32):
        r[2 * i, 2 * i + 1] = -1.0
        r[2 * i + 1, 2 * i] = 1.0
    r2 = np.zeros((128, 128), dtype=np.float32)
    r2[0:64, 0:64] = r
    r2[64:128, 64:128] = r
    r2t = np.ascontiguousarray(r2.T)

    # mt[c, k] = -MASKB where k > c (strictly above the block diagonal)
    rr = np.arange(128)[:, None]  # c
    cc = np.arange(128)[None, :]  # k
    maskt = np.where(cc > rr, -MASKB, 0.0).astype(np.float32)
    mask01t = (rr <= cc).astype(np.float32)
    identt = np.eye(128, dtype=np.float32)
    return cosp, sinp, r2t, maskt, mask01t, identt


def _in_maps(x, Wqkv, Wproj):
    import ml_dtypes

    cosp, sinp, r2t, maskt, mask01t, identt = _host_tables()
    maps = []
    for c in range(NCORES):
        b, g = divmod(c, 4)
        xTc = np.ascontiguousarray(x[b].T).astype(np.float32)
        wq = Wqkv[:, g * 256:(g + 1) * 256]
        wk = Wqkv[:, C + g * 256: C + (g + 1) * 256]
        wvv = Wqkv[:, 2 * C + g * 256: 2 * C + (g + 1) * 256]
        wqkc = np.ascontiguousarray(
            np.concatenate([wq, wk], axis=1), dtype=np.float32
        )
        maps.append(
            {
                "xT": xTc,
                "wqk": wqkc,
                "wv": np.ascontiguousarray(wvv, dtype=np.float32),
                "wpj": np.ascontiguousarray(
                    Wproj[g * 256:(g + 1) * 256, :], dtype=np.float32
                ),
                "r2t": r2t.astype(ml_dtypes.bfloat16),
                "cosp": cosp,
                "sinp": sinp,
                "maskt": maskt.astype(ml_dtypes.bfloat16),
                "mask01t": mask01t.astype(ml_dtypes.bfloat16),
                "identt": identt.astype(ml_dtypes.bfloat16),
            }
        )
    return maps


def _assemble(results):
    out = np.zeros((B, T, C), dtype=np.float32)
    for c in range(NCORES):
        b = c // 4
        out[b] += results[c]["y"]
    return out


def kernel(x, Wqkv, Wproj):
    x = np.asarray(x, dtype=np.float32)
    Wqkv = np.asarray(Wqkv, dtype=np.float32)
    Wproj = np.asarray(Wproj, dtype=np.float32)
    nc = _get_nc()
    maps = _in_maps(x, Wqkv, Wproj)
    res = run_bass_kernel_spmd(nc, maps, core_ids=list(range(NCORES)))
    return _assemble(res.results)


# revision 8
# speedup vs baseline: 46.1338x; 1.0024x over previous
"""Causal self-attention with RoPE on 8 trn2 NeuronCores — v2.

Full inputs -> full output. Sharding: data-parallel on batch (2) x
tensor-parallel on heads (4 heads/core). Each core computes qkv projections,
RoPE, causal attention for its 4 heads, and a partial output projection
(row-parallel slice of Wproj); the host sums the 4 partials per batch.

v2 changes vs v1 (all aimed at PE idle-gap removal; PE is the roofline):
  - causal diagonal masking folded into the scores PSUM accumulation as a
    constant bias matmul (mt^T @ I adds -2^17.58 above the diagonal), so
    exp() output feeds the AV matmul directly: no GPSIMD mask op, no
    Pool->PE dependency chain.
  - RoPE runs in-place in one PSUM bank per pack: proj matmuls accumulate
    ps; DVE writes tsin = ps*sin (SBUF); DVE overwrites ps *= cos; the PE
    rotation matmul accumulates R@tsin onto ps (start=False); one DVE copy
    emits the bf16 pack. Frees 2 PSUM banks vs the v1 rp scheme and
    decouples Q/K-proj PSUM recycling from V/output-proj tiles.
  - PSUM pools: ps (2 banks) / sc (4) / avE+avO (2), with pv/pj sharing
    the ps pool's rotation. No pool oversubscription.
  - the scores stream runs one kt block ahead of the AV matmuls, so each
    exp() executes in the shadow of the next block's scores.
  - filler weaving is deadline-paced globally: next-chunk projections
    finish within the current chunk, while each finished chunk's output
    projection spreads over the next THREE chunks (ytsb[j] isn't rewritten
    until the next pass), so thin-filler chunks aren't starved and chunk
    boundaries don't burst.
  - PSUM-side copies are routed explicitly and balanced across ACT/DVE
    (half the output-tile copies on ACT, the rest plus the V copies on DVE):
    'any' routing let copies clump on ACT and stall the sc-bank recycle.
  - both heads' V tiles live in one 256-wide padded block with a shared
    ones column, so a single regular-AP copy per V t-tile fills both AV
    stationaries (16 copies/pass instead of 32).
  - softmax skips max-subtraction (scores are ~N(0,1); exp stays well
    inside bf16 range) and gets the denominator from an appended
    ones-column in V.
"""

import math
from functools import lru_cache

import numpy as np

import concourse.bacc as bacc
import concourse.bass as bass
import concourse.mybir as mybir
import concourse.tile as tile
from concourse.bass import ds, ts
from concourse.bass_utils import run_bass_kernel_spmd

B, T, C = 2, 2048, 1024
H, DH = 16, 64
HL = 4  # heads per core
NCORES = 8
ROPE_BASE = 10000.0
MASKB = 196608.0  # causal bias; exactly representable in bf16; /8 -> -24576

F32 = mybir.dt.float32
F32R = mybir.dt.float32r
BF16 = mybir.dt.bfloat16
AF = mybir.ActivationFunctionType
ALU = mybir.AluOpType

TCH = 512  # T chunk (free dim of projection / q chunk of attention)
NJ = T // TCH  # 4
NKT = T // 128  # 16 k tiles


def _build_nc(reps=1, variant=""):
    nc = bacc.Bacc("TRN2", target_bir_lowering=False, debug=False, num_devices=NCORES)

    xT = nc.dram_tensor("xT", [C, T], F32R, kind="ExternalInput")
    wqk = nc.dram_tensor("wqk", [C, 512], F32R, kind="ExternalInput")
    wv = nc.dram_tensor("wv", [C, 256], F32R, kind="ExternalInput")
    wpj = nc.dram_tensor("wpj", [256, C], F32R, kind="ExternalInput")
    r2t = nc.dram_tensor("r2t", [128, 128], BF16, kind="ExternalInput")
    cosp = nc.dram_tensor("cosp", [128, T], F32, kind="ExternalInput")
    sinp = nc.dram_tensor("sinp", [128, T], F32, kind="ExternalInput")
    maskt = nc.dram_tensor("maskt", [128, 128], BF16, kind="ExternalInput")
    mask01t = nc.dram_tensor("mask01t", [128, 128], BF16, kind="ExternalInput")
    identt = nc.dram_tensor("identt", [128, 128], BF16, kind="ExternalInput")
    y = nc.dram_tensor("y", [T, C], F32, kind="ExternalOutput")
    # scratch rows for the denominator partition-broadcast round-trip
    rscr = nc.dram_tensor("rscr", [NJ, 2, 2, TCH], F32, kind="Internal")

    xT_r = xT.rearrange("(co p) t -> p co t", p=128)
    wqk_r = wqk.rearrange("(co p) m -> p co m", p=128)
    wv_r = wv.rearrange("(co p) m -> p co m", p=128)
    wpj_r = wpj.rearrange("(cc p) n -> p cc n", p=128)

    with tile.TileContext(nc) as tc:
        with (
            tc.tile_pool(name="const", bufs=1) as const,
            tc.tile_pool(name="persist", bufs=1) as persist,
            tc.tile_pool(name="work", bufs=3) as work,
            tc.tile_pool(name="expool", bufs=12) as expool,
            tc.tile_pool(name="small", bufs=3) as small,
            tc.tile_pool(name="outst", bufs=8) as outst,
            tc.tile_pool(name="psP", bufs=2, space="PSUM") as psP,
            tc.tile_pool(name="psS", bufs=2, space="PSUM") as psS,
            tc.tile_pool(name="psV", bufs=1, space="PSUM") as psV,
        ):
            # ---- constants ----
            wqk_sb = const.tile([128, 8, 512], F32R)
            wv_sb = const.tile([128, 8, 256], F32R)
            wpj_sb = const.tile([128, 2, 1024], F32R)
            r2t_sb = const.tile([128, 128], BF16)
            mask_sb = const.tile([128, 128], BF16)
            mask01_sb = const.tile([128, 128], BF16)
            ident_sb = const.tile([128, 128], BF16)
            cos_sb = const.tile([128, T], F32)
            sin_sb = const.tile([128, T], F32)
            # first-wave DMAs: only what the first matmuls touch
            for _cc in range(8):
                nc.sync.dma_start(wqk_sb[:, _cc, :], wqk_r[:, _cc, :])
            nc.sync.dma_start(r2t_sb[:], r2t[:])

            # ---- persistent activations ----
            qt_packs = [
                persist.tile([128, T], BF16, tag=f"qt{p}", name=f"qt{p}")
                for p in range(2)
            ]
            kt_packs = [
                persist.tile([128, T], BF16, tag=f"kt{p}", name=f"kt{p}")
                for p in range(2)
            ]
            # merged V for both heads of each pair, 256-wide blocks:
            # [v_even 0:64 | ones@64 | zeros 65:128 | v_odd 128:192 | pad].
            # avE lhsT = [0:65] (v|ones), avO lhsT = [64:192] (ones|zeros|v):
            # the ones column is shared, and a SINGLE copy per V t-tile fills
            # both heads (dest = (pp, parity, 0:64) of the (2,128)-factored
            # view; parity 1 lands at col 128).
            vv = persist.tile([128, NKT, 2, 256], BF16, tag="vv")
            ytsb = persist.tile([128, 2, T], F32R, tag="ytsb")

            nc.gpsimd.memset(vv[:, :, :, 64], 1.0)
            nc.gpsimd.memset(vv[:, :, :, 65:128], 0.0)

            def gen_A(j):
                """Projections + RoPE + V for chunk j. Yields between groups."""
                xt = work.tile([128, 8, TCH], F32R, tag="xt", name=f"xt{j}")
                for _cc in range(8):
                    nc.sync.dma_start(xt[:, _cc, :], xT_r[:, _cc, ts(j, TCH)])
                yield
                for pk in range(4):  # Q pack0, Q pack1, K pack0, K pack1
                    ps = psP.tile([128, TCH], F32, tag="mm", name=f"ps{j}{pk}")
                    for cc in range(4):
                        nc.tensor.matmul(
                            ps[:],
                            wqk_sb[:, cc, ts(pk, 128)],
                            xt[:, cc, :],
                            start=(cc == 0),
                            stop=False,
                        )
                    yield
                    for cc in range(4, 8):
                        nc.tensor.matmul(
                            ps[:],
                            wqk_sb[:, cc, ts(pk, 128)],
                            xt[:, cc, :],
                            start=False,
                            stop=(cc == 7),
                        )
                    yield
                    # rot(q*sin) == rot(q)*sin: sin-multiply straight off PSUM;
                    # the three DVE ops are emitted back-to-back so no other
                    # DVE work lands inside the ps-recycle chain
                    tsin = work.tile([128, TCH], BF16, tag="tsin", name=f"tsn{j}{pk}")
                    nc.vector.tensor_tensor(
                        tsin[:], ps[:], sin_sb[:, ts(j, TCH)], ALU.mult
                    )
                    # in-place: ps *= cos, then accumulate R @ tsin onto it
                    nc.vector.tensor_tensor(
                        ps[:], ps[:], cos_sb[:, ts(j, TCH)], ALU.mult
                    )
                    nc.tensor.matmul(
                        ps[:], r2t_sb[:], tsin[:],
                        start=False, stop=True, skip_group_check=True,
                    )
                    dest = (qt_packs + kt_packs)[pk]
                    nc.vector.tensor_copy(dest[:, ts(j, TCH)], ps[:])
                    yield
                for i in range(4):  # V t-tiles of this chunk
                    kt_i = j * 4 + i
                    pv = psP.tile([128, 256], F32, tag="mm", name=f"pv{j}{i}")
                    for cc in range(4):
                        nc.tensor.matmul(
                            pv[:],
                            xt[:, cc, ts(i, 128)],
                            wv_sb[:, cc, :],
                            start=(cc == 0),
                            stop=False,
                        )
                    yield
                    for cc in range(4, 8):
                        nc.tensor.matmul(
                            pv[:],
                            xt[:, cc, ts(i, 128)],
                            wv_sb[:, cc, :],
                            start=False,
                            stop=(cc == 7),
                        )
                    pv4 = pv[:].rearrange("p (a b d) -> p a b d", a=2, b=2)
                    vvd = vv[:, kt_i].rearrange("p a (b c) -> p a b c", b=2)
                    nc.vector.tensor_copy(vvd[:, :, :, 0:64], pv4[:])
                    yield

            def gen_B(j):
                """Attention for q chunk j. Yields between kt blocks.

                Scores run one block ahead of the AV matmuls so each exp()
                executes in the shadow of the next block's scores.
                """
                for pp in range(2):  # head pairs (2pp, 2pp+1)
                    KT = kt_packs[pp]
                    QT = qt_packs[pp]
                    n_kt = 4 * j + 4
                    avE = psV.tile([65, TCH], F32, tag="avE", name=f"avE{j}{pp}")
                    avO = psV.tile([128, TCH], F32, tag="avO", name=f"avO{j}{pp}")
                    exs = {}

                    def emit_sc(kt):
                        d = kt - 4 * j
                        col0 = max(d, 0) * 128
                        sc = psS.tile(
                            [128, 2, TCH], F32, tag="sc", name=f"sc{j}{pp}{kt}"
                        )
                        if d >= 0 and "poolmask" in variant:
                            qsl = ds(j * TCH + col0, TCH - col0)
                            nc.tensor.matmul(
                                sc[:, 0, col0:], KT[0:64, ts(kt, 128)],
                                QT[0:64, qsl],
                                start=True, stop=True, tile_position=(0, 0),
                            )
                            nc.tensor.matmul(
                                sc[:, 1, col0:], KT[64:128, ts(kt, 128)],
                                QT[64:128, qsl],
                                start=True, stop=True, tile_position=(64, 0),
                            )
                        elif d >= 0:
                            # causal bias on the diagonal 128-block, then the
                            # two 64-contract head matmuls accumulate onto it
                            asl = slice(col0, col0 + 128)
                            qasl = ds(j * TCH + col0, 128)
                            for h in range(2):
                                nc.tensor.matmul(
                                    sc[:, h, asl],
                                    mask_sb[:],
                                    ident_sb[:],
                                    start=True, stop=False,
                                    skip_group_check=True,
                                )
                                nc.tensor.matmul(
                                    sc[:, h, asl],
                                    KT[64 * h:64 * h + 64, ts(kt, 128)],
                                    QT[64 * h:64 * h + 64, qasl],
                                    start=False, stop=True,
                                    skip_group_check=True,
                                    tile_position=(64 * h, 0),
                                )
                            if col0 + 128 < TCH:
                                bsl = ds(j * TCH + col0 + 128, TCH - col0 - 128)
                                nc.tensor.matmul(
                                    sc[:, 0, col0 + 128:],
                                    KT[0:64, ts(kt, 128)], QT[0:64, bsl],
                                    start=True, stop=True, tile_position=(0, 0),
                                )
                                nc.tensor.matmul(
                                    sc[:, 1, col0 + 128:],
                                    KT[64:128, ts(kt, 128)], QT[64:128, bsl],
                                    start=True, stop=True, tile_position=(64, 0),
                                )
                        else:
                            qsl = ds(j * TCH, TCH)
                            nc.tensor.matmul(
                                sc[:, 0, :], KT[0:64, ts(kt, 128)], QT[0:64, qsl],
                                start=True, stop=True, tile_position=(0, 0),
                            )
                            nc.tensor.matmul(
                                sc[:, 1, :], KT[64:128, ts(kt, 128)], QT[64:128, qsl],
                                start=True, stop=True, tile_position=(64, 0),
                            )
                        ex = expool.tile(
                            [128, 2, TCH], BF16, tag="ex", name=f"ex{j}{pp}{kt}"
                        )
                        if "noexp" not in variant:
                            nc.scalar.activation(
                                ex[:, :, col0:], sc[:, :, col0:], AF.Exp,
                                scale=1.0 / math.sqrt(DH),
                            )
                        if d >= 0 and "poolmask" in variant:
                            msl = slice(col0, col0 + 128)
                            nc.gpsimd.tensor_tensor(
                                ex[:, :, msl],
                                ex[:, :, msl],
                                mask01_sb[:, None, :].to_broadcast((128, 2, 128)),
                                ALU.mult,
                            )
                        exs[kt] = (ex, col0)

                    def emit_av(kt):
                        ex, col0 = exs.pop(kt)
                        st = kt == 0
                        sp = kt == n_kt - 1
                        nc.tensor.matmul(
                            avE[:, col0:], vv[:, kt, pp, 0:65], ex[:, 0, col0:],
                            start=st, stop=sp,
                        )
                        nc.tensor.matmul(
                            avO[:, col0:], vv[:, kt, pp, 64:192], ex[:, 1, col0:],
                            start=st, stop=sp,
                        )

                    for kt in range(n_kt + 1):
                        if kt < n_kt:
                            emit_sc(kt)
                        if kt > 0:
                            emit_av(kt - 1)
                        yield
                    # denominators: even head at avE row 64, odd at avO row 0.
                    # reciprocal the raw rows on DVE, DMA-broadcast via DRAM,
                    # then the two normalize multiplies.
                    rec = small.tile([128, TCH], F32, tag="rec", name=f"rc{j}{pp}")
                    nc.vector.reciprocal(rec[64:65, :], avE[64:65, :])
                    nc.vector.reciprocal(rec[0:1, :], avO[0:1, :])
                    nc.sync.dma_start(rscr[j, pp, 0:1, :], rec[64:65, :])
                    nc.sync.dma_start(rscr[j, pp, 1:2, :], rec[0:1, :])
                    bc = small.tile([128, TCH], F32, tag="bc", name=f"bc{j}{pp}")
                    nc.sync.dma_start(
                        bc[0:64, :],
                        rscr[j, pp, 0, :][None, :].to_broadcast((64, TCH)),
                    )
                    nc.sync.dma_start(
                        bc[64:128, :],
                        rscr[j, pp, 1, :][None, :].to_broadcast((64, TCH)),
                    )
                    nc.vector.tensor_tensor(
                        ytsb[0:64, pp, ts(j, TCH)], avE[0:64, :], bc[0:64, :],
                        ALU.mult,
                    )
                    nc.vector.tensor_tensor(
                        ytsb[64:128, pp, ts(j, TCH)], avO[64:128, :], bc[64:128, :],
                        ALU.mult,
                    )
                    yield

            def gen_C(j):
                """Output projection for chunk j. Yields between tiles."""
                for i in range(4):
                    qt_i = j * 4 + i
                    for co in range(2):
                        pj = psP.tile([128, TCH], F32, tag="mm", name=f"pj{j}{i}{co}")
                        for cc in range(2):
                            nc.tensor.matmul(
                                pj[:],
                                ytsb[:, cc, ts(qt_i, 128)],
                                wpj_sb[:, cc, ts(co, TCH)],
                                start=(cc == 0),
                                stop=(cc == 1),
                            )
                        ot = outst.tile([128, TCH], F32, tag="ot", name=f"ot{j}{i}{co}")
                        if co == 0:
                            nc.vector.tensor_copy(ot[:], pj[:])
                        else:
                            nc.scalar.copy(ot[:], pj[:])
                        nc.sync.dma_start(y[ts(qt_i, 128), ts(co, TCH)], ot[:])
                        yield

            def drain(g):
                for _ in g:
                    pass

            # software-pipelined emission across the chunk sequence.
            chunks = [(r, j) for r in range(reps) for j in range(NJ)]
            a0 = gen_A(chunks[0][1])
            next(a0)  # xt chunk-0 DMAs
            nc.sync.dma_start(cos_sb[:, ts(0, TCH)], cosp[:, ts(0, TCH)])
            nc.sync.dma_start(sin_sb[:, ts(0, TCH)], sinp[:, ts(0, TCH)])
            for _ in range(12):  # the four Q/K pack groups
                next(a0)
            for _cc in range(8):
                nc.sync.dma_start(wv_sb[:, _cc, :], wv_r[:, _cc, :])
            for _c in range(1, NJ):
                nc.sync.dma_start(cos_sb[:, ts(_c, TCH)], cosp[:, ts(_c, TCH)])
                nc.sync.dma_start(sin_sb[:, ts(_c, TCH)], sinp[:, ts(_c, TCH)])
            nc.sync.dma_start(mask_sb[:], maskt[:])
            nc.sync.dma_start(mask01_sb[:], mask01t[:])
            nc.sync.dma_start(ident_sb[:], identt[:])
            for _cc in range(2):
                nc.sync.dma_start(wpj_sb[:, _cc, :], wpj_r[:, _cc, :])
            drain(a0)

            # deadline-paced weaving: filler generators (projections for the
            # next chunk, output projection for finished chunks) are drained
            # at a per-block rate that spreads each one evenly to its
            # deadline, so thin-filler chunks aren't starved and chunk ends
            # don't dump PE-work bursts while ACT idles.
            class Fill:
                def __init__(self, gen, steps, deadline):
                    self.gen = gen
                    self.rem = steps
                    self.deadline = deadline  # chunk idx it must finish in
                    self.credit = 0.0

                def step(self):
                    try:
                        next(self.gen)
                    except StopIteration:
                        self.rem = 0
                        return
                    self.rem -= 1

            def blocks_of(idx):
                return 2 * (4 * chunks[idx][1] + 6)

            fills = []
            for idx, (r, j) in enumerate(chunks):
                if idx + 1 < len(chunks):
                    fills.append(Fill(gen_A(chunks[idx + 1][1]), 21, idx))
                n_blocks = blocks_of(idx)
                pair_blocks = n_blocks // 2
                done = 0
                for _ in gen_B(j):
                    done += 1
                    w = 1.0
                    for f in sorted(fills, key=lambda f: f.deadline):
                        if f.rem <= 0:
                            continue
                        togo = n_blocks - done
                        for k in range(idx + 1, min(f.deadline, len(chunks) - 1) + 1):
                            togo += blocks_of(k)
                        f.credit += w * f.rem / max(togo, 1)
                        while f.credit >= 1.0 and f.rem > 0:
                            f.step()
                            f.credit -= 1.0
                for f in fills:
                    if f.deadline <= idx:
                        while f.rem > 0:
                            f.step()
                        drain(f.gen)
                fills = [f for f in fills if f.rem > 0]
                lastc = min(idx + 3, len(chunks) - 1)
                fills.append(Fill(gen_C(j), 8, lastc))
            for f in fills:
                drain(f.gen)

    nc.compile()
    return nc


@lru_cache(maxsize=8)
def _get_nc(reps=1, variant=""):
    return _build_nc(reps, variant)


def _host_tables():
    dh = DH
    invf = 1.0 / (ROPE_BASE ** (np.arange(0, dh, 2, dtype=np.float64) / dh))
    t = np.arange(T, dtype=np.float64)
    freqs = np.outer(t, invf)  # [T, 32]
    cos_td = np.repeat(np.cos(freqs), 2, axis=1)  # [T, 64]
    sin_td = np.repeat(np.sin(freqs), 2, axis=1)
    cosp = np.tile(cos_td.T, (2, 1)).astype(np.float32)  # [128, T]
    sinp = np.tile(sin_td.T, (2, 1)).astype(np.float32)

    r = np.zeros((64, 64), dtype=np.float32)
    for i in range(32):
        r[2 * i, 2 * i + 1] = -1.0
        r[2 * i + 1, 2 * i] = 1.0
    r2 = np.zeros((128, 128), dtype=np.float32)
    r2[0:64, 0:64] = r
    r2[64:128, 64:128] = r
    r2t = np.ascontiguousarray(r2.T)

    # mt[c, k] = -MASKB where k > c (strictly above the block diagonal)
    rr = np.arange(128)[:, None]  # c
    cc = np.arange(128)[None, :]  # k
    maskt = np.where(cc > rr, -MASKB, 0.0).astype(np.float32)
    mask01t = (rr <= cc).astype(np.float32)
    identt = np.eye(128, dtype=np.float32)
    return cosp, sinp, r2t, maskt, mask01t, identt


def _in_maps(x, Wqkv, Wproj):
    import ml_dtypes

    cosp, sinp, r2t, maskt, mask01t, identt = _host_tables()
    maps = []
    for c in range(NCORES):
        b, g = divmod(c, 4)
        xTc = np.ascontiguousarray(x[b].T).astype(np.float32)
        wq = Wqkv[:, g * 256:(g + 1) * 256]
        wk = Wqkv[:, C + g * 256: C + (g + 1) * 256]
        wvv = Wqkv[:, 2 * C + g * 256: 2 * C + (g + 1) * 256]
        wqkc = np.ascontiguousarray(
            np.concatenate([wq, wk], axis=1), dtype=np.float32
        )
        maps.append(
            {
                "xT": xTc,
                "wqk": wqkc,
                "wv": np.ascontiguousarray(wvv, dtype=np.float32),
                "wpj": np.ascontiguousarray(
                    Wproj[g * 256:(g + 1) * 256, :], dtype=np.float32
                ),
                "r2t": r2t.astype(ml_dtypes.bfloat16),
                "cosp": cosp,
                "sinp": sinp,
                "maskt": maskt.astype(ml_dtypes.bfloat16),
                "mask01t": mask01t.astype(ml_dtypes.bfloat16),
                "identt": identt.astype(ml_dtypes.bfloat16),
            }
        )
    return maps


def _assemble(results):
    out = np.zeros((B, T, C), dtype=np.float32)
    for c in range(NCORES):
        b = c // 4
        out[b] += results[c]["y"]
    return out


def kernel(x, Wqkv, Wproj):
    x = np.asarray(x, dtype=np.float32)
    Wqkv = np.asarray(Wqkv, dtype=np.float32)
    Wproj = np.asarray(Wproj, dtype=np.float32)
    nc = _get_nc()
    maps = _in_maps(x, Wqkv, Wproj)
    res = run_bass_kernel_spmd(nc, maps, core_ids=list(range(NCORES)))
    return _assemble(res.results)
